# revision 24
# baseline (speedup 1.0000x reference)
"""Trainium2 Bass kernel for multi-head attention (B=4, H=8, L=2048, dim=512).

Sharding: 8 cores = 4 batches x 2 sequence halves. Each core computes the
full attention output for one batch's 1024-query half (all 8 heads), using
K/V over the full 2048-key sequence; the output projection contracts the
full hidden dim locally, so no cross-core communication is needed.

v5 design (per-core), 3-engine balance:
  - Scores matmuls row-pack the two heads of a pair (64-wide contraction
    each), issue-ordered A-qc0, B-qc0, A-qc1, B-qc1 so the row-disjoint
    pairs co-stream in the PE array; attn@V col-packs the two heads
    (stationaries in disjoint column groups, so weight loads overlap the
    other head's streaming matmuls).
  - exp split across engines: track A exact on ScalarE, issued as two
    per-qc calls so the next tile's scores unblock halfway; track B
    mostly via a Schraudolph bit-trick on the DVE: q is pre-scaled by
    1024*log2(e) on the host, so one tensor_scalar add of the fp16
    exponent bias with an int16 convert, bitcast to fp16, yields exp with
    ~1.8% rms error (final output rel err ~7e-3, gate 2e-2); 4 of 16
    track-B tiles run exactly on ScalarE for engine balance.
  - Softmax denominators: track A accumulates on the PE via an all-ones
    [128,64] stationary matmul into dedicated PSUM banks (broadcast
    across 64 partitions for free); track B sums via progressive fp16
    accumulators (DVE, plus a light GpSimd chain for the ScalarE-exp'd
    tiles), finished by a per-pair ones-matmul partition-sum into the
    same den banks at partitions 64:128 so normalization multiplies stay
    partition-aligned.
  - K projections for pair m+1 are interleaved into pair m's kt loop;
    phase 1 proper only computes Q, pair-0 K, and V^T.
  - PSUM: scores 2x[P,QL] (4) + po (2) + denA/denB-qc banks (2) = 8.
"""
import numpy as np

import concourse.bass as bass
import concourse.tile as tile
from concourse import bacc, mybir
from concourse.bass_utils import run_bass_kernel_spmd

F16 = mybir.dt.float16
F32 = mybir.dt.float32
I16 = mybir.dt.int16
P = 128
D = 512          # model dim
L = 2048         # full sequence (keys)
QL = 1024        # per-core query length
H = 8            # heads
C = 64           # head dim
HID = 512        # H * C
DC = D // P      # 4 contraction chunks
KT = L // P      # 16 key tiles
N = 512          # matmul free-dim chunk
QC = QL // N     # 2 query chunks
LC = L // N      # 4 key free-dim chunks
LOG2E = 1.4426950408889634
SCALE = C ** -0.5
QSCALE = SCALE * 1024.0 * LOG2E      # folded into wq on the host
EXPSC = float(np.log(2.0) / 1024.0)  # ScalarE exp scale undoing QSCALE
BSHIFT = 15.0 * 1024.0 - 60.0        # Schraudolph fp16 exponent bias
EXP = mybir.ActivationFunctionType.Exp
IDENT = mybir.ActivationFunctionType.Identity
# track-B kt tiles whose exp runs (exactly) on ScalarE instead of the DVE;
# kt 15 on ScalarE so the end-of-pair denominator tail starts fast.
SCALAR_B_KTS = (1, 5, 9, 15)
GP_B_KTS = (1, 5, 9)  # ScalarE-exp'd B tiles summed on GpSimd (minus 15)


def emit(nc, tc, x, wq, wk, wv, wo, bias, out):
    import contextlib
    ctx = contextlib.ExitStack()
    with ctx:
        # ---- pools -----------------------------------------------------
        consts = ctx.enter_context(tc.tile_pool(name="consts", bufs=1))
        qkv = ctx.enter_context(tc.tile_pool(name="qkv", bufs=1))
        ph1 = ctx.enter_context(tc.tile_pool(name="ph1", bufs=1))
        atAp = ctx.enter_context(tc.tile_pool(name="atAp", bufs=4))
        atBp = ctx.enter_context(tc.tile_pool(name="atBp", bufs=3))
        # ScalarE-exp'd B tiles live longer (read by the GpSimd accumulator
        # a few kt later), so they get their own slots
        atBSp = ctx.enter_context(tc.tile_pool(name="atBSp", bufs=2))
        accp = ctx.enter_context(tc.tile_pool(name="accp", bufs=2))
        t1p = ctx.enter_context(tc.tile_pool(name="t1p", bufs=4))
        rcpp = ctx.enter_context(tc.tile_pool(name="rcpp", bufs=2))
        rcp16p = ctx.enter_context(tc.tile_pool(name="rcp16p", bufs=2))
        otup = ctx.enter_context(tc.tile_pool(name="otup", bufs=2))
        outp = ctx.enter_context(tc.tile_pool(name="outp", bufs=2))
        # PSUM: scores 2x[P,QL] (4 banks) + po (2) + den0/den1 (2)
        pps = ctx.enter_context(tc.tile_pool(name="pps", bufs=4, space="PSUM"))
        ppo = ctx.enter_context(tc.tile_pool(name="ppo", bufs=1, space="PSUM"))
        pd0 = ctx.enter_context(tc.tile_pool(name="pd0", bufs=1, space="PSUM"))
        pd1 = ctx.enter_context(tc.tile_pool(name="pd1", bufs=1, space="PSUM"))

        # ---- persistent SBUF ------------------------------------------
        wo_sb = consts.tile([P, DC, HID], F16)
        bias_sb = consts.tile([P, DC], F32)
        ones_sb = consts.tile([P, C], F16)
        nc.vector.memset(ones_sb[:], 1.0)

        q_sb = qkv.tile([P, DC, QL], F16)
        k_sb = qkv.tile([P, DC, L], F16)
        vt_sb = qkv.tile([P, KT, HID], F16)
        ot_sb = qkv.tile([P, DC, QL], F16)

        # ---- loads -----------------------------------------------------
        x_sb = ph1.tile([P, DC, L], F16)
        wq_sb = ph1.tile([P, DC, HID], F16)
        wk_sb = ph1.tile([P, DC, HID], F16)
        wv_sb = ph1.tile([P, DC, HID], F16)
        xr = x.rearrange("(a p) n -> p a n", p=P)
        nc.sync.dma_start(out=wq_sb[:], in_=wq.rearrange("(a p) n -> p a n", p=P))
        nc.sync.dma_start(out=x_sb[:, :, 0:N], in_=xr[:, :, 0:N])
        nc.sync.dma_start(out=x_sb[:, :, N:QL], in_=xr[:, :, N:QL])
        nc.sync.dma_start(out=wk_sb[:], in_=wk.rearrange("(a p) n -> p a n", p=P))
        nc.sync.dma_start(out=x_sb[:, :, QL:L], in_=xr[:, :, QL:L])
        nc.sync.dma_start(out=wv_sb[:], in_=wv.rearrange("(a p) n -> p a n", p=P))
        nc.sync.dma_start(out=wo_sb[:], in_=wo.rearrange("(a p) n -> p a n", p=P))
        nc.sync.dma_start(out=bias_sb[:], in_=bias)

        _prj = [0]

        def proj_psum():
            _prj[0] += 1
            return pps.tile([P, N], F32, tag="ps", name=f"prj{_prj[0]}")

        cp = [0]

        def proj_copy(dst, src):
            # split PSUM->SBUF projection copies across ScalarE and DVE
            cp[0] += 1
            if cp[0] % 2 == 0:
                nc.scalar.copy(dst, src)
            else:
                nc.vector.tensor_copy(dst, src)

        def q_proj(m, qc):
            ps = proj_psum()
            for dc in range(DC):
                nc.tensor.matmul(
                    ps[:],
                    lhsT=wq_sb[:, dc, m * P:(m + 1) * P],
                    rhs=x_sb[:, dc, qc * N:(qc + 1) * N],
                    start=(dc == 0), stop=(dc == DC - 1),
                )
            proj_copy(q_sb[:, m, qc * N:(qc + 1) * N], ps[:])

        def k_proj(m, lc):
            ps = proj_psum()
            for dc in range(DC):
                nc.tensor.matmul(
                    ps[:],
                    lhsT=wk_sb[:, dc, m * P:(m + 1) * P],
                    rhs=x_sb[:, dc, lc * N:(lc + 1) * N],
                    start=(dc == 0), stop=(dc == DC - 1),
                )
            proj_copy(k_sb[:, m, lc * N:(lc + 1) * N], ps[:])

        def vt_proj(kt):
            # V^T: [k, hc] (x stationary)
            ps = proj_psum()
            for dc in range(DC):
                nc.tensor.matmul(
                    ps[:],
                    lhsT=x_sb[:, dc, kt * P:(kt + 1) * P],
                    rhs=wv_sb[:, dc, :],
                    start=(dc == 0), stop=(dc == DC - 1),
                )
            proj_copy(vt_sb[:, kt, :], ps[:])

        # phase 1 proper: Q first, then K and V interleaved by chunk
        for m in range(DC):
            for qc in range(QC):
                q_proj(m, qc)
        for lc in range(LC):
            for mm_ in range(DC):
                k_proj(mm_, lc)
            for kt_ in range(4 * lc, 4 * lc + 4):
                vt_proj(kt_)

        # ---- phase 2: attention, one head pair (2m, 2m+1) at a time ----
        for m in range(DC):
            po = ppo.tile([P, QL], F32, tag="po", name=f"po{m}")
            den0 = pd0.tile([P, N], F32, tag="d0", name=f"den0_{m}")
            den1 = pd1.tile([P, N], F32, tag="d1", name=f"den1_{m}")
            accV = accp.tile([P, QL], F16, tag="accV", name=f"accV{m}")
            accG = accp.tile([P, QL], F16, tag="accG", name=f"accG{m}")
            firstV = firstG = None
            nV = nG = 0
            atB_last = None
            at_hist = {}

            def attnv_den(j):
                atAj, atBj = at_hist.pop(j)
                # attn @ V: col-packed pair (A cols 0:63, B cols 64:127)
                for qc in range(QC):
                    nc.tensor.matmul(
                        po[0:C, qc * N:(qc + 1) * N],
                        lhsT=vt_sb[:, j, (2 * m) * C:(2 * m + 1) * C],
                        rhs=atAj[:, qc * N:(qc + 1) * N],
                        start=(j == 0), stop=(j == KT - 1),
                        tile_position=(0, 0), skip_group_check=True,
                    )
                    nc.tensor.matmul(
                        po[C:P, qc * N:(qc + 1) * N],
                        lhsT=vt_sb[:, j, (2 * m + 1) * C:(2 * m + 2) * C],
                        rhs=atBj[:, qc * N:(qc + 1) * N],
                        start=(j == 0), stop=(j == KT - 1),
                        tile_position=(0, C), skip_group_check=True,
                    )
                # A denominator: ones-matmul accumulation (broadcast rows)
                for qc, den_t in ((0, den0), (1, den1)):
                    nc.tensor.matmul(
                        den_t[0:C, :],
                        lhsT=ones_sb[:],
                        rhs=atAj[:, qc * N:(qc + 1) * N],
                        start=(j == 0), stop=(j == KT - 1),
                        tile_position=(0, 0), skip_group_check=True,
                    )
            for kt in range(KT):
                psA = [pps.tile([P, N], F32, tag="ps", name=f"psA{kt}_{i}")
                       for i in range(QC)]
                psB = [pps.tile([P, N], F32, tag="ps", name=f"psB{kt}_{i}")
                       for i in range(QC)]
                at_A = atAp.tile([P, QL], F16, tag="at")
                on_scalar = kt in SCALAR_B_KTS
                if on_scalar:
                    atB = atBSp.tile([P, QL], I16, tag="atbs")
                else:
                    atB = atBp.tile([P, QL], I16, tag="atb")
                atB16 = atB[:].bitcast(F16)
                # scores: interleave row-packed pairs for PE co-streaming
                for qc in range(QC):
                    nc.tensor.matmul(
                        psA[qc][:],
                        lhsT=k_sb[0:C, m, kt * P:(kt + 1) * P],
                        rhs=q_sb[0:C, m, qc * N:(qc + 1) * N],
                        start=True, stop=True, tile_position=(0, 0),
                    )
                    nc.tensor.matmul(
                        psB[qc][:],
                        lhsT=k_sb[C:P, m, kt * P:(kt + 1) * P],
                        rhs=q_sb[C:P, m, qc * N:(qc + 1) * N],
                        start=True, stop=True, tile_position=(C, 0),
                    )
                # exp: track A exact on ScalarE; track B mostly Schraudolph
                # on the DVE.  At kt 15, B goes first so the end-of-pair
                # denominator tail starts as early as possible.
                if on_scalar and kt == KT - 1:
                    for qc in range(QC):
                        nc.scalar.activation(atB16[:, qc * N:(qc + 1) * N],
                                             psB[qc][:], EXP, scale=EXPSC)
                    for qc in range(QC):
                        nc.scalar.activation(at_A[:, qc * N:(qc + 1) * N],
                                             psA[qc][:], EXP, scale=EXPSC)
                else:
                    for qc in range(QC):
                        nc.scalar.activation(at_A[:, qc * N:(qc + 1) * N],
                                             psA[qc][:], EXP, scale=EXPSC)
                    if on_scalar:
                        for qc in range(QC):
                            nc.scalar.activation(
                                atB16[:, qc * N:(qc + 1) * N],
                                psB[qc][:], EXP, scale=EXPSC)
                    else:
                        for qc in range(QC):
                            nc.vector.tensor_scalar_add(
                                atB[:, qc * N:(qc + 1) * N], psB[qc][:],
                                BSHIFT)
                # attn @ V and the A-denominator accumulate deferred by
                # two kt so the new pair's PSUM groups start only after
                # the previous pair's recip/copy chains released the banks
                at_hist[kt] = (at_A, atB16)
                if kt >= 2:
                    attnv_den(kt - 2)
                # B denominator: progressive accumulators
                if kt == KT - 1:
                    atB_last = atB16
                elif on_scalar:
                    if firstG is None:
                        firstG = atB16
                    elif nG == 0:
                        nc.gpsimd.tensor_add(accG[:], firstG, atB16)
                        nG = 1
                    else:
                        nc.gpsimd.tensor_add(accG[:], accG[:], atB16)
                else:
                    if firstV is None:
                        firstV = atB16
                    elif nV == 0:
                        nc.vector.tensor_add(accV[:], firstV, atB16)
                        nV = 1
                    else:
                        nc.vector.tensor_add(accV[:], accV[:], atB16)

            attnv_den(KT - 2)
            attnv_den(KT - 1)

            # ---- end-of-pair tail ------------------------------------
            t1a = t1p.tile([P, QL], F16, tag="t1", name=f"t1a_{m}")
            t1b = t1p.tile([P, QL], F16, tag="t1", name=f"t1b_{m}")
            nc.vector.tensor_add(t1a[:], accV[:], accG[:])
            nc.vector.tensor_add(t1b[:], t1a[:], atB_last)
            for qc, den_t in ((0, den0), (1, den1)):
                nc.tensor.matmul(
                    den_t[C:P, :],
                    lhsT=ones_sb[:],
                    rhs=t1b[:, qc * N:(qc + 1) * N],
                    start=True, stop=True,
                    tile_position=(0, C), skip_group_check=True,
                )
            # free the po banks quickly, then normalize
            otu = otup.tile([P, QL], F16, tag="otu", name=f"otu{m}")
            nc.scalar.copy(otu[:, 0:N], po[:, 0:N])
            nc.vector.tensor_copy(otu[:, N:QL], po[:, N:QL])
            for qc, den_t in ((0, den0), (1, den1)):
                rc32 = rcpp.tile([P, N], F32, tag="rc32", name=f"rc32_{m}{qc}")
                nc.vector.reciprocal_approx_fast(out=rc32[:], in_=den_t[:])
                rc16 = rcp16p.tile([P, N], F16, tag="rc16", name=f"rc16_{m}{qc}")
                nc.vector.tensor_copy(rc16[:], rc32[:])
                for half in range(2):
                    nc.vector.tensor_mul(
                        ot_sb[half * C:(half + 1) * C, m, qc * N:(qc + 1) * N],
                        otu[half * C:(half + 1) * C, qc * N:(qc + 1) * N],
                        rc16[half * C:(half + 1) * C, :],
                    )

        # ---- phase 3: output projection + bias -------------------------
        # Batched over the 4 PSUM slots: each batch's first three
        # contraction terms (pairs 0-2, long since ready) are emitted
        # before any pair-3 term, so they overlap the last pair's
        # normalization tail instead of serializing behind it.
        groups = [(mo, qc) for mo in range(DC) for qc in range(QC)]
        for b0 in (0, 4):
            batch = groups[b0:b0 + 4]
            pss = []
            for mo, qc in batch:
                ps = pps.tile([P, N], F32, tag="ps", name=f"po3_{mo}_{qc}")
                pss.append(ps)
                for mh in range(DC - 1):
                    nc.tensor.matmul(
                        ps[:],
                        lhsT=wo_sb[:, mh, mo * P:(mo + 1) * P],
                        rhs=ot_sb[:, mh, qc * N:(qc + 1) * N],
                        start=(mh == 0), stop=False,
                    )
            for (mo, qc), ps in zip(batch, pss):
                nc.tensor.matmul(
                    ps[:],
                    lhsT=wo_sb[:, DC - 1, mo * P:(mo + 1) * P],
                    rhs=ot_sb[:, DC - 1, qc * N:(qc + 1) * N],
                    start=False, stop=True,
                )
                ob = outp.tile([P, N], F32, tag="ob")
                if (mo + qc) % 2 == 0:
                    nc.vector.tensor_scalar_add(ob[:], ps[:], bias_sb[:, mo:mo + 1])
                else:
                    nc.scalar.activation(ob[:], ps[:], IDENT,
                                         bias=bias_sb[:, mo:mo + 1])
                nc.sync.dma_start(
                    out=out[mo * P:(mo + 1) * P, qc * N:(qc + 1) * N], in_=ob[:]
                )


def build():
    nc = bacc.Bacc("TRN2", target_bir_lowering=False, debug=False)
    x = nc.dram_tensor("x", [D, L], F16, kind="ExternalInput").ap()
    wq = nc.dram_tensor("wq", [D, HID], F16, kind="ExternalInput").ap()
    wk = nc.dram_tensor("wk", [D, HID], F16, kind="ExternalInput").ap()
    wv = nc.dram_tensor("wv", [D, HID], F16, kind="ExternalInput").ap()
    wo = nc.dram_tensor("wo", [HID, D], F16, kind="ExternalInput").ap()
    bias = nc.dram_tensor("bias", [P, DC], F32, kind="ExternalInput").ap()
    out = nc.dram_tensor("out", [D, QL], F32, kind="ExternalOutput").ap()
    with tile.TileContext(nc) as tc:
        emit(nc, tc, x, wq, wk, wv, wo, bias, out)
    nc.compile()
    return nc


_NC_CACHE = None


def _get_nc():
    global _NC_CACHE
    if _NC_CACHE is None:
        _NC_CACHE = build()
    return _NC_CACHE


def make_in_maps(x, w_qkv, w_out, b_out):
    """Host-side sharding: returns the 8 per-core input dicts."""
    f16 = np.float16
    wq_t = np.ascontiguousarray((w_qkv[0:HID] * QSCALE).T).astype(f16)
    wk_t = np.ascontiguousarray(w_qkv[HID:2 * HID].T).astype(f16)
    wv_t = np.ascontiguousarray(w_qkv[2 * HID:3 * HID].T).astype(f16)
    wo_t = np.ascontiguousarray(w_out.T).astype(f16)
    bias = np.ascontiguousarray(b_out.reshape(DC, P).T).astype(np.float32)
    in_maps = []
    for core in range(8):
        b, halfq = core // 2, core % 2
        # rotate so this core's query half sits at columns 0:QL; key order
        # is irrelevant (softmax sums over all keys).
        x_rot = np.roll(x[b], -halfq * QL, axis=1).astype(f16)
        in_maps.append({
            "x": np.ascontiguousarray(x_rot),
            "wq": wq_t, "wk": wk_t, "wv": wv_t, "wo": wo_t,
            "bias": bias,
        })
    return in_maps


def assemble(results):
    out = np.zeros((4, D, L), np.float32)
    for core in range(8):
        b, halfq = core // 2, core % 2
        out[b][:, halfq * QL:(halfq + 1) * QL] = results[core]["out"]
    return out


def kernel(x, w_qkv, w_out, b_out):
    x = np.asarray(x, np.float32)
    w_qkv = np.asarray(w_qkv, np.float32)
    w_out = np.asarray(w_out, np.float32)
    b_out = np.asarray(b_out, np.float32)
    nc = _get_nc()
    in_maps = make_in_maps(x, w_qkv, w_out, b_out)
    res = run_bass_kernel_spmd(nc, in_maps, core_ids=list(range(8)))
    return assemble(res.results)
